# revision 1
# baseline (speedup 1.0000x reference)
import sys
import numpy as np

sys.path.insert(0, '/opt/trn_rl_repo')

import concourse.bass as bass
import concourse.bacc as bacc
import concourse.tile as tile
from concourse import mybir
from concourse.bass_utils import run_bass_kernel_spmd
from contextlib import ExitStack

F32 = mybir.dt.float32
F32R = mybir.dt.float32r

B, S, HID = 2, 4096, 4096
NH, HD = 16, 256
RD = 64
THETA = 10000.0
NKMAX = 8          # max k-chunks of 512 per q-tile row
NEG = -1.0e30

_cached = {}


def _build_program():
    nc = bacc.Bacc("TRN2", target_bir_lowering=False, debug=False, num_devices=8)
    hiddenT = nc.declare_dram_parameter("hiddenT", [HID, S], F32R, isOutput=False)
    wqkvT = nc.declare_dram_parameter("wqkvT", [HID, 3072], F32R, isOutput=False)
    woutTp = nc.declare_dram_parameter("woutTp", [HID, 1024], F32R, isOutput=False)
    cs_e = nc.declare_dram_parameter("cs", [S, 32], F32, isOutput=False)
    sn_e = nc.declare_dram_parameter("sn", [S, 32], F32, isOutput=False)
    msk_e = nc.declare_dram_parameter("msk", [128, 4, 512], F32, isOutput=False)
    id_e = nc.declare_dram_parameter("ident", [128, 128], F32R, isOutput=False)
    out_e = nc.declare_dram_parameter("out", [S, 1024], F32, isOutput=True)

    Copy = mybir.ActivationFunctionType.Copy
    Exp = mybir.ActivationFunctionType.Exp
    AX = mybir.AxisListType.X

    with tile.TileContext(nc) as tc:
        with tc.tile_pool(name="dram", bufs=1, space="DRAM") as dram:
            qs = dram.tile([S, 1024], F32R)
            ks = dram.tile([S, 1024], F32R)
            vs = dram.tile([S, 1024], F32R)
            at_h = [dram.tile([256, S], F32R, name=f"at{j}") for j in range(4)]
            gt_h = [dram.tile([1024, S], F32R, name=f"gt{j}") for j in range(4)]

            # ---------------- phase 1: QKV projection + RoPE ----------------
            with ExitStack() as s1:
                wpool = s1.enter_context(tc.tile_pool(name="wq", bufs=1))
                hpool = s1.enter_context(tc.tile_pool(name="hid", bufs=2))
                evpool = s1.enter_context(tc.tile_pool(name="ev", bufs=4))
                cpool = s1.enter_context(tc.tile_pool(name="cspool", bufs=2))
                tpool = s1.enter_context(tc.tile_pool(name="ropetmp", bufs=4))
                pq = s1.enter_context(tc.tile_pool(name="pq", bufs=2, space="PSUM"))
                hviews = hiddenT.ap().rearrange("(ho p) s -> p ho s", p=128)
                for wb in range(3):
                    wt = []
                    for h in range(32):
                        w_t = wpool.tile([128, 1024], F32R, name=f"w{h}", tag=f"w{h}")
                        nc.sync.dma_start(
                            out=w_t,
                            in_=wqkvT.ap()[h * 128:(h + 1) * 128,
                                           wb * 1024:(wb + 1) * 1024])
                        wt.append(w_t)
                    for st in range(32):
                        hs = hpool.tile([128, 32, 128], F32R, name="hs")
                        nc.sync.dma_start(
                            out=hs, in_=hviews[:, :, st * 128:(st + 1) * 128])
                        if wb < 2:
                            ct = cpool.tile([128, 32], F32, name="ct")
                            snt = cpool.tile([128, 32], F32, name="snt")
                            nc.sync.dma_start(
                                out=ct, in_=cs_e.ap()[st * 128:(st + 1) * 128, :])
                            nc.sync.dma_start(
                                out=snt, in_=sn_e.ap()[st * 128:(st + 1) * 128, :])
                        for oc in range(2):
                            ps = pq.tile([128, 512], F32, name="qkps")
                            for h in range(32):
                                nc.tensor.matmul(
                                    ps, hs[:, h, :],
                                    wt[h][:, oc * 512:(oc + 1) * 512],
                                    start=(h == 0), stop=(h == 31))
                            ev = evpool.tile([128, 512], F32R, name="ev")
                            if wb < 2:
                                for hb in range(2):
                                    b0 = hb * 256
                                    x1 = ps[:, b0 + 0:b0 + 64:2]
                                    x2 = ps[:, b0 + 1:b0 + 65:2]
                                    ta = tpool.tile([128, 32], F32, name="ta")
                                    tb = tpool.tile([128, 32], F32, name="tb")
                                    nc.vector.tensor_mul(ta, x1, ct)
                                    nc.vector.tensor_mul(tb, x2, snt)
                                    nc.vector.tensor_sub(ev[:, b0:b0 + 32], ta, tb)
                                    tc2 = tpool.tile([128, 32], F32, name="tc2")
                                    td = tpool.tile([128, 32], F32, name="td")
                                    nc.vector.tensor_mul(tc2, x2, ct)
                                    nc.vector.tensor_mul(td, x1, snt)
                                    nc.vector.tensor_add(
                                        ev[:, b0 + 32:b0 + 64], tc2, td)
                                    nc.scalar.activation(
                                        ev[:, b0 + 64:b0 + 256],
                                        ps[:, b0 + 64:b0 + 256], Copy)
                            else:
                                nc.scalar.activation(ev, ps, Copy)
                            dst = (qs, ks, vs)[wb]
                            nc.sync.dma_start(
                                out=dst[st * 128:(st + 1) * 128,
                                        oc * 512:(oc + 1) * 512],
                                in_=ev)

            # ---------------- phase 2: attention per head + gather ----------
            with ExitStack() as s2:
                kv = s2.enter_context(tc.tile_pool(name="kv", bufs=1))
                scp = s2.enter_context(tc.tile_pool(name="scp", bufs=1))
                small = s2.enter_context(tc.tile_pool(name="small", bufs=4))
                ptp = s2.enter_context(tc.tile_pool(name="ptp", bufs=3))
                consts = s2.enter_context(tc.tile_pool(name="consts", bufs=1))
                pst = s2.enter_context(tc.tile_pool(name="pst", bufs=2, space="PSUM"))
                pso = s2.enter_context(tc.tile_pool(name="pso", bufs=2, space="PSUM"))
                idt = consts.tile([128, 128], F32R)
                nc.sync.dma_start(out=idt, in_=id_e.ap())
                mskt = consts.tile([128, 4, 512], F32)
                nc.sync.dma_start(out=mskt, in_=msk_e.ap())
                vviews = vs.rearrange("(st p) o -> p st o", p=128)
                for h in range(4):
                    KT = [kv.tile([128, S], F32R, name=f"kt{d}", tag=f"kt{d}")
                          for d in range(2)]
                    QT = [kv.tile([128, S], F32R, name=f"qt{d}", tag=f"qt{d}")
                          for d in range(2)]
                    for st in range(32):
                        kin = ptp.tile([128, 256], F32R, name="kin")
                        nc.sync.dma_start(
                            out=kin, in_=ks[st * 128:(st + 1) * 128,
                                            h * 256:(h + 1) * 256])
                        qin = ptp.tile([128, 256], F32R, name="qin")
                        nc.sync.dma_start(
                            out=qin, in_=qs[st * 128:(st + 1) * 128,
                                            h * 256:(h + 1) * 256])
                        for d in range(2):
                            tpk = pst.tile([128, 128], F32R, name="tprs", tag="tprs")
                            nc.tensor.transpose(tpk, kin[:, d * 128:(d + 1) * 128], idt)
                            nc.vector.tensor_copy(
                                KT[d][:, st * 128:(st + 1) * 128], tpk)
                            tpq = pst.tile([128, 128], F32R, name="tprs", tag="tprs")
                            nc.tensor.transpose(tpq, qin[:, d * 128:(d + 1) * 128], idt)
                            nc.vector.tensor_copy(
                                QT[d][:, st * 128:(st + 1) * 128], tpq)
                    vt = kv.tile([128, 32, 256], F32R, name="vt", tag="vt")
                    nc.sync.dma_start(
                        out=vt, in_=vviews[:, :, h * 256:(h + 1) * 256])
                    for qi in range(32):
                        nk = qi // 4 + 1
                        srow = scp.tile([128, S], F32, name="srow", tag="srow")
                        prow = scp.tile([128, S], F32R, name="prow", tag="prow")
                        for kc in range(nk):
                            pss = pst.tile([128, 512], F32, name="spsum", tag="spsum")
                            for d in range(2):
                                nc.tensor.matmul(
                                    pss, QT[d][:, qi * 128:(qi + 1) * 128],
                                    KT[d][:, kc * 512:(kc + 1) * 512],
                                    start=(d == 0), stop=(d == 1))
                            if kc == nk - 1:
                                nc.vector.tensor_add(
                                    srow[:, kc * 512:(kc + 1) * 512], pss,
                                    mskt[:, qi % 4, :])
                            else:
                                nc.scalar.activation(
                                    srow[:, kc * 512:(kc + 1) * 512], pss, Copy)
                        nmx = small.tile([128, 1], F32, name="nmx")
                        nc.vector.reduce_max(nmx, srow[:, 0:nk * 512],
                                             axis=AX, negate=True)
                        bia = small.tile([128, 1], F32, name="bia")
                        nc.vector.tensor_scalar_mul(bia, nmx, 1.0 / 16.0)
                        sums = small.tile([128, NKMAX], F32, name="sums")
                        for kc in range(nk):
                            nc.scalar.activation(
                                prow[:, kc * 512:(kc + 1) * 512],
                                srow[:, kc * 512:(kc + 1) * 512], Exp,
                                bias=bia, scale=1.0 / 16.0,
                                accum_out=sums[:, kc:kc + 1])
                        ssum = small.tile([128, 1], F32, name="ssum")
                        nc.vector.reduce_sum(ssum, sums[:, 0:nk], axis=AX)
                        rinv = small.tile([128, 1], F32, name="rinv")
                        nc.vector.reciprocal(rinv, ssum)
                        pot = pso.tile([128, 256], F32, name="opsum")
                        for kc in range(nk):
                            for t4 in range(4):
                                g = kc * 4 + t4
                                tpp = pst.tile([128, 128], F32R,
                                               name="tprs", tag="tprs")
                                nc.tensor.transpose(
                                    tpp, prow[:, g * 128:(g + 1) * 128], idt)
                                pts = ptp.tile([128, 128], F32R, name="pts")
                                nc.vector.tensor_copy(pts, tpp)
                                nc.tensor.matmul(
                                    pot, pts, vt[:, g, :],
                                    start=(g == 0), stop=(g == nk * 4 - 1))
                        att = ptp.tile([128, 256], F32R, name="att")
                        nc.vector.tensor_scalar_mul(att, pot, rinv)
                        for d in range(2):
                            tpa = pst.tile([128, 128], F32R, name="tprs", tag="tprs")
                            nc.tensor.transpose(
                                tpa, att[:, d * 128:(d + 1) * 128], idt)
                            ats = ptp.tile([128, 128], F32R, name="ats")
                            nc.vector.tensor_copy(ats, tpa)
                            nc.sync.dma_start(
                                out=at_h[h][d * 128:(d + 1) * 128,
                                            qi * 128:(qi + 1) * 128],
                                in_=ats)
                    nc.gpsimd.collective_compute(
                        "AllGather", mybir.AluOpType.bypass,
                        replica_groups=[[0, 1, 2, 3], [4, 5, 6, 7]],
                        ins=[at_h[h][:]], outs=[gt_h[h][:]])

            # ---------------- phase 3: output projection --------------------
            with ExitStack() as s3:
                wo = s3.enter_context(tc.tile_pool(name="wo", bufs=1))
                ga = s3.enter_context(tc.tile_pool(name="ga", bufs=2))
                ob = s3.enter_context(tc.tile_pool(name="ob", bufs=3))
                pout = s3.enter_context(tc.tile_pool(name="pout", bufs=2, space="PSUM"))
                wot = []
                for hh in range(32):
                    w_o = wo.tile([128, 1024], F32R, name=f"wo{hh}", tag=f"wo{hh}")
                    nc.sync.dma_start(
                        out=w_o, in_=woutTp.ap()[hh * 128:(hh + 1) * 128, :])
                    wot.append(w_o)
                gviews = [g.rearrange("(ho p) s -> p ho s", p=128) for g in gt_h]
                for st in range(32):
                    acb = [ga.tile([128, 8, 128], F32R, name=f"acb{j}", tag=f"acb{j}")
                           for j in range(4)]
                    for j in range(4):
                        nc.sync.dma_start(
                            out=acb[j],
                            in_=gviews[j][:, :, st * 128:(st + 1) * 128])
                    for oc in range(2):
                        po2 = pout.tile([128, 512], F32, name="po2")
                        for j in range(4):
                            for ht in range(8):
                                nc.tensor.matmul(
                                    po2, acb[j][:, ht, :],
                                    wot[j * 8 + ht][:, oc * 512:(oc + 1) * 512],
                                    start=(j == 0 and ht == 0),
                                    stop=(j == 3 and ht == 7))
                        osb = ob.tile([128, 512], F32, name="osb")
                        nc.scalar.activation(osb, po2, Copy)
                        nc.sync.dma_start(
                            out=out_e.ap()[st * 128:(st + 1) * 128,
                                           oc * 512:(oc + 1) * 512],
                            in_=osb)

    nc.compile()
    return nc


def kernel(hidden_states, position_ids, Wqkv, Wout):
    hidden_states = np.asarray(hidden_states, dtype=np.float32)
    position_ids = np.asarray(position_ids)
    Wqkv = np.asarray(Wqkv, dtype=np.float32)
    Wout = np.asarray(Wout, dtype=np.float32)

    if "nc" not in _cached:
        _cached["nc"] = _build_program()
    nc = _cached["nc"]

    inv_freq = (1.0 / (THETA ** (np.arange(0, RD, 2, dtype=np.float64) / RD))
                ).astype(np.float32)
    ident = np.eye(128, dtype=np.float32)
    rr = np.arange(128)[:, None]
    ccol = np.arange(512)[None, :]
    msk = np.stack([np.where(ccol <= 128 * p + rr, 0.0, NEG)
                    for p in range(4)], axis=1).astype(np.float32)  # [128,4,512]

    in_maps = []
    for c in range(8):
        b, r = c // 4, c % 4
        heads = list(range(4 * r, 4 * r + 4))
        hiddenT = np.ascontiguousarray(hidden_states[b].T)
        rows = []
        for sec in range(3):  # q, k, v sections of Wqkv
            for h in heads:
                rows.append(Wqkv[sec * HID + h * HD:sec * HID + (h + 1) * HD])
        wqkvT = np.ascontiguousarray(np.concatenate(rows, axis=0).T)
        hperm = np.array([(4 * cc + j) * HD + d
                          for j in range(4) for cc in range(4)
                          for d in range(HD)])
        woutTp = np.ascontiguousarray(Wout[r * 1024:(r + 1) * 1024][:, hperm].T)
        pos = position_ids[b].astype(np.float32)
        fr = pos[:, None] * inv_freq[None, :]
        in_maps.append({
            "hiddenT": hiddenT, "wqkvT": wqkvT, "woutTp": woutTp,
            "cs": np.cos(fr).astype(np.float32),
            "sn": np.sin(fr).astype(np.float32),
            "msk": msk, "ident": ident,
        })

    res = run_bass_kernel_spmd(nc, in_maps, list(range(8))).results
    out = np.empty((B, S, HID), dtype=np.float32)
    for b in range(B):
        for r in range(4):
            out[b][:, r * 1024:(r + 1) * 1024] = res[4 * b + r]["out"]
    return out


# revision 5
# speedup vs baseline: 1.2064x; 1.2064x over previous
import sys
import numpy as np

sys.path.insert(0, '/opt/trn_rl_repo')

import concourse.bass as bass
import concourse.bacc as bacc
import concourse.tile as tile
from concourse import mybir
from concourse.bass_utils import run_bass_kernel_spmd
from contextlib import ExitStack

F32 = mybir.dt.float32
F32R = mybir.dt.float32r

B, S, HID = 2, 4096, 4096
NH, HD = 16, 256
RD = 64
THETA = 10000.0
NKMAX = 8          # max k-chunks of 512 per q-tile row
NEG = -1.0e30

_cached = {}


def _build_program():
    nc = bacc.Bacc("TRN2", target_bir_lowering=False, debug=False, num_devices=8)
    # hidden, transposed and swizzled host-side into contiguous 2MB col-blocks:
    # hsw[st] = hiddenT[:, st*128:(st+1)*128]
    hiddenT = nc.declare_dram_parameter("hiddenT", [32, HID, 128], F32R,
                                        isOutput=False)
    wqkvT = nc.declare_dram_parameter("wqkvT", [HID, 3072], F32R, isOutput=False)
    woutTp = nc.declare_dram_parameter("woutTp", [HID, 1024], F32R, isOutput=False)
    cs_e = nc.declare_dram_parameter("cs", [S, 32], F32, isOutput=False)
    sn_e = nc.declare_dram_parameter("sn", [S, 32], F32, isOutput=False)
    msk_e = nc.declare_dram_parameter("msk", [128, 4, 512], F32, isOutput=False)
    id_e = nc.declare_dram_parameter("ident", [128, 128], F32R, isOutput=False)
    out_e = nc.declare_dram_parameter("out", [S, 1024], F32, isOutput=True)

    Copy = mybir.ActivationFunctionType.Copy
    Exp = mybir.ActivationFunctionType.Exp
    AX = mybir.AxisListType.X

    with tile.TileContext(nc) as tc:
        with tc.tile_pool(name="dram", bufs=1, space="DRAM") as dram:
            qs = dram.tile([S, 1024], F32R)
            ks = dram.tile([S, 1024], F32R)
            vs = dram.tile([S, 1024], F32R)
            at_h = [dram.tile([256, S], F32R, name=f"at{j}") for j in range(4)]
            gt_h = [dram.tile([1024, S], F32R, name=f"gt{j}") for j in range(4)]

            # ---------------- phase 1: QKV projection + RoPE ----------------
            with ExitStack() as s1:
                wpool = s1.enter_context(tc.tile_pool(name="wq", bufs=1))
                hpool = s1.enter_context(tc.tile_pool(name="hid", bufs=2))
                evpool = s1.enter_context(tc.tile_pool(name="ev", bufs=4))
                cpool = s1.enter_context(tc.tile_pool(name="cspool", bufs=2))
                tpool = s1.enter_context(tc.tile_pool(name="ropetmp", bufs=4))
                pq = s1.enter_context(tc.tile_pool(name="pq", bufs=2, space="PSUM"))
                hviews = hiddenT.ap().rearrange("t (ho p) s -> t p ho s", p=128)
                for wb in range(3):
                    wt = []
                    for h in range(32):
                        w_t = wpool.tile([128, 1024], F32R, name=f"w{h}", tag=f"w{h}")
                        nc.sync.dma_start(
                            out=w_t,
                            in_=wqkvT.ap()[h * 128:(h + 1) * 128,
                                           wb * 1024:(wb + 1) * 1024])
                        wt.append(w_t)
                    for st in range(32):
                        hs = hpool.tile([128, 32, 128], F32R, name="hs")
                        nc.sync.dma_start(out=hs, in_=hviews[st])
                        if wb < 2:
                            ct = cpool.tile([128, 32], F32, name="ct")
                            snt = cpool.tile([128, 32], F32, name="snt")
                            nc.sync.dma_start(
                                out=ct, in_=cs_e.ap()[st * 128:(st + 1) * 128, :])
                            nc.sync.dma_start(
                                out=snt, in_=sn_e.ap()[st * 128:(st + 1) * 128, :])
                        for oc in range(2):
                            ps = pq.tile([128, 512], F32, name="qkps")
                            for h in range(32):
                                nc.tensor.matmul(
                                    ps, hs[:, h, :],
                                    wt[h][:, oc * 512:(oc + 1) * 512],
                                    start=(h == 0), stop=(h == 31))
                            ev = evpool.tile([128, 512], F32R, name="ev")
                            if wb < 2:
                                for hb in range(2):
                                    b0 = hb * 256
                                    x1 = ps[:, b0 + 0:b0 + 64:2]
                                    x2 = ps[:, b0 + 1:b0 + 65:2]
                                    ta = tpool.tile([128, 32], F32, name="ta")
                                    tb = tpool.tile([128, 32], F32, name="tb")
                                    nc.vector.tensor_mul(ta, x1, ct)
                                    nc.vector.tensor_mul(tb, x2, snt)
                                    nc.vector.tensor_sub(ev[:, b0:b0 + 32], ta, tb)
                                    tc2 = tpool.tile([128, 32], F32, name="tc2")
                                    td = tpool.tile([128, 32], F32, name="td")
                                    nc.vector.tensor_mul(tc2, x2, ct)
                                    nc.vector.tensor_mul(td, x1, snt)
                                    nc.vector.tensor_add(
                                        ev[:, b0 + 32:b0 + 64], tc2, td)
                                    nc.scalar.activation(
                                        ev[:, b0 + 64:b0 + 256],
                                        ps[:, b0 + 64:b0 + 256], Copy)
                            else:
                                nc.scalar.activation(ev, ps, Copy)
                            dst = (qs, ks, vs)[wb]
                            nc.sync.dma_start(
                                out=dst[st * 128:(st + 1) * 128,
                                        oc * 512:(oc + 1) * 512],
                                in_=ev)

            # ---------------- phase 2: attention per head + gather ----------
            with ExitStack() as s2:
                kv = s2.enter_context(tc.tile_pool(name="kv", bufs=1))
                scp = s2.enter_context(tc.tile_pool(name="scp", bufs=1))
                small = s2.enter_context(tc.tile_pool(name="small", bufs=4))
                ptp = s2.enter_context(tc.tile_pool(name="ptp", bufs=3))
                consts = s2.enter_context(tc.tile_pool(name="consts", bufs=1))
                pst = s2.enter_context(tc.tile_pool(name="pst", bufs=2, space="PSUM"))
                pso = s2.enter_context(tc.tile_pool(name="pso", bufs=2, space="PSUM"))
                idt = consts.tile([128, 128], F32R)
                nc.sync.dma_start(out=idt, in_=id_e.ap())
                mskt = consts.tile([128, 4, 512], F32)
                nc.sync.dma_start(out=mskt, in_=msk_e.ap())
                vviews = vs.rearrange("(st p) o -> p st o", p=128)
                for h in range(4):
                    KT = [kv.tile([128, S], F32R, name=f"kt{d}", tag=f"kt{d}")
                          for d in range(2)]
                    QT = [kv.tile([128, S], F32R, name=f"qt{d}", tag=f"qt{d}")
                          for d in range(2)]
                    for st in range(32):
                        kin = ptp.tile([128, 256], F32R, name="kin")
                        nc.sync.dma_start(
                            out=kin, in_=ks[st * 128:(st + 1) * 128,
                                            h * 256:(h + 1) * 256])
                        qin = ptp.tile([128, 256], F32R, name="qin")
                        nc.sync.dma_start(
                            out=qin, in_=qs[st * 128:(st + 1) * 128,
                                            h * 256:(h + 1) * 256])
                        for d in range(2):
                            tpk = pst.tile([128, 128], F32R, name="tprs", tag="tprs")
                            nc.tensor.transpose(tpk, kin[:, d * 128:(d + 1) * 128], idt)
                            nc.vector.tensor_copy(
                                KT[d][:, st * 128:(st + 1) * 128], tpk)
                            tpq = pst.tile([128, 128], F32R, name="tprs", tag="tprs")
                            nc.tensor.transpose(tpq, qin[:, d * 128:(d + 1) * 128], idt)
                            nc.vector.tensor_copy(
                                QT[d][:, st * 128:(st + 1) * 128], tpq)
                    vt = kv.tile([128, 32, 256], F32R, name="vt", tag="vt")
                    nc.sync.dma_start(
                        out=vt, in_=vviews[:, :, h * 256:(h + 1) * 256])
                    for qi in range(32):
                        nk = qi // 4 + 1
                        srow = scp.tile([128, S], F32, name="srow", tag="srow")
                        prow = scp.tile([128, S], F32R, name="prow", tag="prow")
                        for kc in range(nk):
                            pss = pst.tile([128, 512], F32, name="spsum", tag="spsum")
                            for d in range(2):
                                nc.tensor.matmul(
                                    pss, QT[d][:, qi * 128:(qi + 1) * 128],
                                    KT[d][:, kc * 512:(kc + 1) * 512],
                                    start=(d == 0), stop=(d == 1))
                            if kc == nk - 1:
                                nc.vector.tensor_add(
                                    srow[:, kc * 512:(kc + 1) * 512], pss,
                                    mskt[:, qi % 4, :])
                            else:
                                nc.scalar.activation(
                                    srow[:, kc * 512:(kc + 1) * 512], pss, Copy)
                        nmx = small.tile([128, 1], F32, name="nmx")
                        nc.vector.reduce_max(nmx, srow[:, 0:nk * 512],
                                             axis=AX, negate=True)
                        bia = small.tile([128, 1], F32, name="bia")
                        nc.vector.tensor_scalar_mul(bia, nmx, 1.0 / 16.0)
                        sums = small.tile([128, NKMAX], F32, name="sums")
                        for kc in range(nk):
                            nc.scalar.activation(
                                prow[:, kc * 512:(kc + 1) * 512],
                                srow[:, kc * 512:(kc + 1) * 512], Exp,
                                bias=bia, scale=1.0 / 16.0,
                                accum_out=sums[:, kc:kc + 1])
                        ssum = small.tile([128, 1], F32, name="ssum")
                        nc.vector.reduce_sum(ssum, sums[:, 0:nk], axis=AX)
                        rinv = small.tile([128, 1], F32, name="rinv")
                        nc.vector.reciprocal(rinv, ssum)
                        pot = pso.tile([128, 256], F32, name="opsum")
                        for kc in range(nk):
                            for t4 in range(4):
                                g = kc * 4 + t4
                                tpp = pst.tile([128, 128], F32R,
                                               name="tprs", tag="tprs")
                                nc.tensor.transpose(
                                    tpp, prow[:, g * 128:(g + 1) * 128], idt)
                                pts = ptp.tile([128, 128], F32R, name="pts")
                                nc.vector.tensor_copy(pts, tpp)
                                nc.tensor.matmul(
                                    pot, pts, vt[:, g, :],
                                    start=(g == 0), stop=(g == nk * 4 - 1))
                        att = ptp.tile([128, 256], F32R, name="att")
                        nc.vector.tensor_scalar_mul(att, pot, rinv)
                        for d in range(2):
                            tpa = pst.tile([128, 128], F32R, name="tprs", tag="tprs")
                            nc.tensor.transpose(
                                tpa, att[:, d * 128:(d + 1) * 128], idt)
                            ats = ptp.tile([128, 128], F32R, name="ats")
                            nc.vector.tensor_copy(ats, tpa)
                            nc.sync.dma_start(
                                out=at_h[h][d * 128:(d + 1) * 128,
                                            qi * 128:(qi + 1) * 128],
                                in_=ats)
                    nc.gpsimd.collective_compute(
                        "AllGather", mybir.AluOpType.bypass,
                        replica_groups=[[0, 1, 2, 3], [4, 5, 6, 7]],
                        ins=[at_h[h][:]], outs=[gt_h[h][:]])

            # ---------------- phase 3: output projection --------------------
            with ExitStack() as s3:
                wo = s3.enter_context(tc.tile_pool(name="wo", bufs=1))
                ga = s3.enter_context(tc.tile_pool(name="ga", bufs=2))
                ob = s3.enter_context(tc.tile_pool(name="ob", bufs=3))
                pout = s3.enter_context(tc.tile_pool(name="pout", bufs=2, space="PSUM"))
                wot = []
                for hh in range(32):
                    w_o = wo.tile([128, 1024], F32R, name=f"wo{hh}", tag=f"wo{hh}")
                    nc.sync.dma_start(
                        out=w_o, in_=woutTp.ap()[hh * 128:(hh + 1) * 128, :])
                    wot.append(w_o)
                gviews = [g.rearrange("(ho p) s -> p ho s", p=128) for g in gt_h]
                for st in range(32):
                    acb = [ga.tile([128, 8, 128], F32R, name=f"acb{j}", tag=f"acb{j}")
                           for j in range(4)]
                    for j in range(4):
                        nc.sync.dma_start(
                            out=acb[j],
                            in_=gviews[j][:, :, st * 128:(st + 1) * 128])
                    for oc in range(2):
                        po2 = pout.tile([128, 512], F32, name="po2")
                        for j in range(4):
                            for ht in range(8):
                                nc.tensor.matmul(
                                    po2, acb[j][:, ht, :],
                                    wot[j * 8 + ht][:, oc * 512:(oc + 1) * 512],
                                    start=(j == 0 and ht == 0),
                                    stop=(j == 3 and ht == 7))
                        osb = ob.tile([128, 512], F32, name="osb")
                        nc.scalar.activation(osb, po2, Copy)
                        nc.sync.dma_start(
                            out=out_e.ap()[st * 128:(st + 1) * 128,
                                           oc * 512:(oc + 1) * 512],
                            in_=osb)

    nc.compile()
    return nc


def kernel(hidden_states, position_ids, Wqkv, Wout):
    hidden_states = np.asarray(hidden_states, dtype=np.float32)
    position_ids = np.asarray(position_ids)
    Wqkv = np.asarray(Wqkv, dtype=np.float32)
    Wout = np.asarray(Wout, dtype=np.float32)

    if "nc" not in _cached:
        _cached["nc"] = _build_program()
    nc = _cached["nc"]

    inv_freq = (1.0 / (THETA ** (np.arange(0, RD, 2, dtype=np.float64) / RD))
                ).astype(np.float32)
    ident = np.eye(128, dtype=np.float32)
    rr = np.arange(128)[:, None]
    ccol = np.arange(512)[None, :]
    msk = np.stack([np.where(ccol <= 128 * p + rr, 0.0, NEG)
                    for p in range(4)], axis=1).astype(np.float32)  # [128,4,512]

    in_maps = []
    for c in range(8):
        b, r = c // 4, c % 4
        heads = list(range(4 * r, 4 * r + 4))
        hiddenT = np.ascontiguousarray(
            hidden_states[b].T.reshape(HID, 32, 128).transpose(1, 0, 2))
        rows = []
        for sec in range(3):  # q, k, v sections of Wqkv
            for h in heads:
                rows.append(Wqkv[sec * HID + h * HD:sec * HID + (h + 1) * HD])
        wqkvT = np.ascontiguousarray(np.concatenate(rows, axis=0).T)
        hperm = np.array([(4 * cc + j) * HD + d
                          for j in range(4) for cc in range(4)
                          for d in range(HD)])
        woutTp = np.ascontiguousarray(Wout[r * 1024:(r + 1) * 1024][:, hperm].T)
        pos = position_ids[b].astype(np.float32)
        fr = pos[:, None] * inv_freq[None, :]
        in_maps.append({
            "hiddenT": hiddenT, "wqkvT": wqkvT, "woutTp": woutTp,
            "cs": np.cos(fr).astype(np.float32),
            "sn": np.sin(fr).astype(np.float32),
            "msk": msk, "ident": ident,
        })

    res = run_bass_kernel_spmd(nc, in_maps, list(range(8))).results
    out = np.empty((B, S, HID), dtype=np.float32)
    for b in range(B):
        for r in range(4):
            out[b][:, r * 1024:(r + 1) * 1024] = res[4 * b + r]["out"]
    return out


# revision 7
# speedup vs baseline: 1.2646x; 1.0482x over previous
import sys
import numpy as np

sys.path.insert(0, '/opt/trn_rl_repo')

import concourse.bass as bass
import concourse.bacc as bacc
import concourse.tile as tile
from concourse import mybir
from concourse.bass_utils import run_bass_kernel_spmd
from contextlib import ExitStack

F32 = mybir.dt.float32
F32R = mybir.dt.float32r

B, S, HID = 2, 4096, 4096
NH, HD = 16, 256
RD = 64
THETA = 10000.0
NKMAX = 8          # max k-chunks of 512 per q-tile row
NEG = -1.0e30

_cached = {}


def _build_program():
    nc = bacc.Bacc("TRN2", target_bir_lowering=False, debug=False, num_devices=8)
    # hidden, transposed and swizzled host-side into contiguous 2MB col-blocks:
    # hsw[st] = hiddenT[:, st*128:(st+1)*128]
    hiddenT = nc.declare_dram_parameter("hiddenT", [32, HID, 128], F32R,
                                        isOutput=False)
    wqkvT = nc.declare_dram_parameter("wqkvT", [HID, 3072], F32R, isOutput=False)
    woutTp = nc.declare_dram_parameter("woutTp", [HID, 1024], F32R, isOutput=False)
    cs_e = nc.declare_dram_parameter("cs", [S, 32], F32, isOutput=False)
    sn_e = nc.declare_dram_parameter("sn", [S, 32], F32, isOutput=False)
    msk_e = nc.declare_dram_parameter("msk", [128, 4, 512], F32, isOutput=False)
    id_e = nc.declare_dram_parameter("ident", [128, 128], F32R, isOutput=False)
    out_e = nc.declare_dram_parameter("out", [S, 1024], F32, isOutput=True)

    Copy = mybir.ActivationFunctionType.Copy
    Exp = mybir.ActivationFunctionType.Exp
    AX = mybir.AxisListType.X

    with tile.TileContext(nc) as tc:
        with tc.tile_pool(name="dram", bufs=1, space="DRAM") as dram:
            qs = dram.tile([S, 1024], F32R)
            ks = dram.tile([S, 1024], F32R)
            vs = dram.tile([S, 1024], F32R)
            at_h = [dram.tile([256, S], F32R, name=f"at{j}") for j in range(4)]
            gt_h = [dram.tile([1024, S], F32R, name=f"gt{j}") for j in range(4)]

            # ---------------- phase 1: QKV projection + RoPE ----------------
            with ExitStack() as s1:
                wpool = s1.enter_context(tc.tile_pool(name="wq", bufs=1))
                hpool = s1.enter_context(tc.tile_pool(name="hid", bufs=2))
                evpool = s1.enter_context(tc.tile_pool(name="ev", bufs=4))
                cpool = s1.enter_context(tc.tile_pool(name="cspool", bufs=2))
                tpool = s1.enter_context(tc.tile_pool(name="ropetmp", bufs=4))
                pq = s1.enter_context(tc.tile_pool(name="pq", bufs=2, space="PSUM"))
                hviews = hiddenT.ap().rearrange("t (ho p) s -> t p ho s", p=128)
                for wb in range(3):
                    wt = []
                    for h in range(32):
                        w_t = wpool.tile([128, 1024], F32R, name=f"w{h}", tag=f"w{h}")
                        nc.sync.dma_start(
                            out=w_t,
                            in_=wqkvT.ap()[h * 128:(h + 1) * 128,
                                           wb * 1024:(wb + 1) * 1024])
                        wt.append(w_t)
                    for st in range(32):
                        hs = hpool.tile([128, 32, 128], F32R, name="hs")
                        nc.sync.dma_start(out=hs, in_=hviews[st])
                        if wb < 2:
                            ct = cpool.tile([128, 32], F32, name="ct")
                            snt = cpool.tile([128, 32], F32, name="snt")
                            nc.sync.dma_start(
                                out=ct, in_=cs_e.ap()[st * 128:(st + 1) * 128, :])
                            nc.sync.dma_start(
                                out=snt, in_=sn_e.ap()[st * 128:(st + 1) * 128, :])
                        for oc in range(2):
                            ps = pq.tile([128, 512], F32, name="qkps")
                            for h in range(32):
                                nc.tensor.matmul(
                                    ps, hs[:, h, :],
                                    wt[h][:, oc * 512:(oc + 1) * 512],
                                    start=(h == 0), stop=(h == 31))
                            ev = evpool.tile([128, 512], F32R, name="ev")
                            if wb < 2:
                                for hb in range(2):
                                    b0 = hb * 256
                                    x1 = ps[:, b0 + 0:b0 + 64:2]
                                    x2 = ps[:, b0 + 1:b0 + 65:2]
                                    ta = tpool.tile([128, 32], F32, name="ta")
                                    tb = tpool.tile([128, 32], F32, name="tb")
                                    nc.vector.tensor_mul(ta, x1, ct)
                                    nc.vector.tensor_mul(tb, x2, snt)
                                    nc.vector.tensor_sub(ev[:, b0:b0 + 32], ta, tb)
                                    tc2 = tpool.tile([128, 32], F32, name="tc2")
                                    td = tpool.tile([128, 32], F32, name="td")
                                    nc.vector.tensor_mul(tc2, x2, ct)
                                    nc.vector.tensor_mul(td, x1, snt)
                                    nc.vector.tensor_add(
                                        ev[:, b0 + 32:b0 + 64], tc2, td)
                                    nc.scalar.activation(
                                        ev[:, b0 + 64:b0 + 256],
                                        ps[:, b0 + 64:b0 + 256], Copy)
                            else:
                                nc.scalar.activation(ev, ps, Copy)
                            dst = (qs, ks, vs)[wb]
                            nc.sync.dma_start(
                                out=dst[st * 128:(st + 1) * 128,
                                        oc * 512:(oc + 1) * 512],
                                in_=ev)

            # ---------------- phase 2: attention per head + gather ----------
            with ExitStack() as s2:
                kv = s2.enter_context(tc.tile_pool(name="kv", bufs=1))
                scp = s2.enter_context(tc.tile_pool(name="scp", bufs=1))
                small = s2.enter_context(tc.tile_pool(name="small", bufs=4))
                ptp = s2.enter_context(tc.tile_pool(name="ptp", bufs=6))
                consts = s2.enter_context(tc.tile_pool(name="consts", bufs=1))
                pst = s2.enter_context(tc.tile_pool(name="pst", bufs=2, space="PSUM"))
                pso = s2.enter_context(tc.tile_pool(name="pso", bufs=2, space="PSUM"))
                idt = consts.tile([128, 128], F32R)
                nc.sync.dma_start(out=idt, in_=id_e.ap())
                mskt = consts.tile([128, 4, 512], F32)
                nc.sync.dma_start(out=mskt, in_=msk_e.ap())
                vviews = vs.rearrange("(st p) o -> p st o", p=128)
                for h in range(4):
                    KT = [kv.tile([128, S], F32R, name=f"kt{d}", tag=f"kt{d}")
                          for d in range(2)]
                    QT = [kv.tile([128, S], F32R, name=f"qt{d}", tag=f"qt{d}")
                          for d in range(2)]
                    for st in range(32):
                        kin = ptp.tile([128, 256], F32R, name="kin")
                        nc.sync.dma_start(
                            out=kin, in_=ks[st * 128:(st + 1) * 128,
                                            h * 256:(h + 1) * 256])
                        qin = ptp.tile([128, 256], F32R, name="qin")
                        nc.sync.dma_start(
                            out=qin, in_=qs[st * 128:(st + 1) * 128,
                                            h * 256:(h + 1) * 256])
                        for d in range(2):
                            tpk = pst.tile([128, 128], F32R, name="tprs", tag="tprs")
                            nc.tensor.transpose(tpk, kin[:, d * 128:(d + 1) * 128], idt)
                            nc.vector.tensor_copy(
                                KT[d][:, st * 128:(st + 1) * 128], tpk)
                            tpq = pst.tile([128, 128], F32R, name="tprs", tag="tprs")
                            nc.tensor.transpose(tpq, qin[:, d * 128:(d + 1) * 128], idt)
                            nc.vector.tensor_copy(
                                QT[d][:, st * 128:(st + 1) * 128], tpq)
                    vt = kv.tile([128, 32, 256], F32R, name="vt", tag="vt")
                    nc.sync.dma_start(
                        out=vt, in_=vviews[:, :, h * 256:(h + 1) * 256])
                    for qi in range(32):
                        nk = qi // 4 + 1
                        srow = scp.tile([128, S], F32, name="srow", tag="srow")
                        prow = scp.tile([128, S], F32R, name="prow", tag="prow")
                        for kc in range(nk):
                            pss = pst.tile([128, 512], F32, name="spsum", tag="spsum")
                            for d in range(2):
                                nc.tensor.matmul(
                                    pss, QT[d][:, qi * 128:(qi + 1) * 128],
                                    KT[d][:, kc * 512:(kc + 1) * 512],
                                    start=(d == 0), stop=(d == 1))
                            if kc == nk - 1:
                                nc.vector.tensor_add(
                                    srow[:, kc * 512:(kc + 1) * 512], pss,
                                    mskt[:, qi % 4, :])
                            else:
                                nc.scalar.activation(
                                    srow[:, kc * 512:(kc + 1) * 512], pss, Copy)
                        nmx = small.tile([128, 1], F32, name="nmx")
                        nc.vector.reduce_max(nmx, srow[:, 0:nk * 512],
                                             axis=AX, negate=True)
                        bia = small.tile([128, 1], F32, name="bia")
                        nc.vector.tensor_scalar_mul(bia, nmx, 1.0 / 16.0)
                        sums = small.tile([128, NKMAX], F32, name="sums")
                        for kc in range(nk):
                            nc.scalar.activation(
                                prow[:, kc * 512:(kc + 1) * 512],
                                srow[:, kc * 512:(kc + 1) * 512], Exp,
                                bias=bia, scale=1.0 / 16.0,
                                accum_out=sums[:, kc:kc + 1])
                        ssum = small.tile([128, 1], F32, name="ssum")
                        nc.vector.reduce_sum(ssum, sums[:, 0:nk], axis=AX)
                        rinv = small.tile([128, 1], F32, name="rinv")
                        nc.vector.reciprocal(rinv, ssum)
                        pot = pso.tile([128, 256], F32, name="opsum")
                        for kc in range(nk):
                            for t4 in range(4):
                                g = kc * 4 + t4
                                tpp = pst.tile([128, 128], F32R,
                                               name="tprs", tag="tprs")
                                nc.tensor.transpose(
                                    tpp, prow[:, g * 128:(g + 1) * 128], idt)
                                pts = ptp.tile([128, 128], F32R, name="pts")
                                nc.vector.tensor_copy(pts, tpp)
                                nc.tensor.matmul(
                                    pot, pts, vt[:, g, :],
                                    start=(g == 0), stop=(g == nk * 4 - 1))
                        att = ptp.tile([128, 256], F32R, name="att")
                        nc.vector.tensor_scalar_mul(att, pot, rinv)
                        for d in range(2):
                            tpa = pst.tile([128, 128], F32R, name="tprs", tag="tprs")
                            nc.tensor.transpose(
                                tpa, att[:, d * 128:(d + 1) * 128], idt)
                            ats = ptp.tile([128, 128], F32R, name="ats")
                            nc.vector.tensor_copy(ats, tpa)
                            nc.sync.dma_start(
                                out=at_h[h][d * 128:(d + 1) * 128,
                                            qi * 128:(qi + 1) * 128],
                                in_=ats)
                    nc.gpsimd.collective_compute(
                        "AllGather", mybir.AluOpType.bypass,
                        replica_groups=[[0, 1, 2, 3], [4, 5, 6, 7]],
                        ins=[at_h[h][:]], outs=[gt_h[h][:]])

            # ---------------- phase 3: output projection --------------------
            with ExitStack() as s3:
                wo = s3.enter_context(tc.tile_pool(name="wo", bufs=1))
                ga = s3.enter_context(tc.tile_pool(name="ga", bufs=2))
                ob = s3.enter_context(tc.tile_pool(name="ob", bufs=3))
                pout = s3.enter_context(tc.tile_pool(name="pout", bufs=2, space="PSUM"))
                wot = []
                for hh in range(32):
                    w_o = wo.tile([128, 1024], F32R, name=f"wo{hh}", tag=f"wo{hh}")
                    nc.sync.dma_start(
                        out=w_o, in_=woutTp.ap()[hh * 128:(hh + 1) * 128, :])
                    wot.append(w_o)
                gviews = [g.rearrange("(ho p) s -> p ho s", p=128) for g in gt_h]
                for st in range(32):
                    acb = [ga.tile([128, 8, 128], F32R, name=f"acb{j}", tag=f"acb{j}")
                           for j in range(4)]
                    for j in range(4):
                        nc.sync.dma_start(
                            out=acb[j],
                            in_=gviews[j][:, :, st * 128:(st + 1) * 128])
                    for oc in range(2):
                        po2 = pout.tile([128, 512], F32, name="po2")
                        for j in range(4):
                            for ht in range(8):
                                nc.tensor.matmul(
                                    po2, acb[j][:, ht, :],
                                    wot[j * 8 + ht][:, oc * 512:(oc + 1) * 512],
                                    start=(j == 0 and ht == 0),
                                    stop=(j == 3 and ht == 7))
                        osb = ob.tile([128, 512], F32, name="osb")
                        nc.scalar.activation(osb, po2, Copy)
                        nc.sync.dma_start(
                            out=out_e.ap()[st * 128:(st + 1) * 128,
                                           oc * 512:(oc + 1) * 512],
                            in_=osb)

    nc.compile()
    return nc


def kernel(hidden_states, position_ids, Wqkv, Wout):
    hidden_states = np.asarray(hidden_states, dtype=np.float32)
    position_ids = np.asarray(position_ids)
    Wqkv = np.asarray(Wqkv, dtype=np.float32)
    Wout = np.asarray(Wout, dtype=np.float32)

    if "nc" not in _cached:
        _cached["nc"] = _build_program()
    nc = _cached["nc"]

    inv_freq = (1.0 / (THETA ** (np.arange(0, RD, 2, dtype=np.float64) / RD))
                ).astype(np.float32)
    ident = np.eye(128, dtype=np.float32)
    rr = np.arange(128)[:, None]
    ccol = np.arange(512)[None, :]
    msk = np.stack([np.where(ccol <= 128 * p + rr, 0.0, NEG)
                    for p in range(4)], axis=1).astype(np.float32)  # [128,4,512]

    in_maps = []
    for c in range(8):
        b, r = c // 4, c % 4
        heads = list(range(4 * r, 4 * r + 4))
        hiddenT = np.ascontiguousarray(
            hidden_states[b].T.reshape(HID, 32, 128).transpose(1, 0, 2))
        rows = []
        for sec in range(3):  # q, k, v sections of Wqkv
            for h in heads:
                rows.append(Wqkv[sec * HID + h * HD:sec * HID + (h + 1) * HD])
        wqkvT = np.ascontiguousarray(np.concatenate(rows, axis=0).T)
        hperm = np.array([(4 * cc + j) * HD + d
                          for j in range(4) for cc in range(4)
                          for d in range(HD)])
        woutTp = np.ascontiguousarray(Wout[r * 1024:(r + 1) * 1024][:, hperm].T)
        pos = position_ids[b].astype(np.float32)
        fr = pos[:, None] * inv_freq[None, :]
        in_maps.append({
            "hiddenT": hiddenT, "wqkvT": wqkvT, "woutTp": woutTp,
            "cs": np.cos(fr).astype(np.float32),
            "sn": np.sin(fr).astype(np.float32),
            "msk": msk, "ident": ident,
        })

    res = run_bass_kernel_spmd(nc, in_maps, list(range(8))).results
    out = np.empty((B, S, HID), dtype=np.float32)
    for b in range(B):
        for r in range(4):
            out[b][:, r * 1024:(r + 1) * 1024] = res[4 * b + r]["out"]
    return out
